# revision 1
# baseline (speedup 1.0000x reference)
"""Trainium2 Bass kernel for nn_BertLexer (weighted layer mix + ragged segment-mean).

Computation (reference):
    w   = softmax(layer_weights)                       # (L,)
    sub = gamma * einsum('l,lbsf->bsf', w, hidden)     # (B,S,F)
    out[b,w,:] = mean over {s : word_ids[b,s]==w} of sub[b,s,:]   (w >= 1)
    out[b,0,:] = mean over all s of sub[b,s,:]

Strategy (8 NeuronCores, data-parallel over B):
  - Each core gets B/8 = 4 sentences.
  - Layer mix on DVE with 3 scalar_tensor_tensor ops per 128x768 chunk via
    ratio folding over weight-sorted layers (a<=b<=c<=d by softmax weight):
    t1 = h_a*(w_a/w_d) + h_d ; t2 = h_b*(w_b/w_c) + h_c ;
    sub = t2*(w_c/w_d) + t1, and the segment matrix absorbs w_d*gamma.
  - Segment mean as an f32r matmul with a host-built per-sentence matrix
    M[s, w] = w_d*gamma/count_w for s in word w's span (M[s,0] =
    w_d*gamma/S for all s), contracting over s on the TensorEngine,
    accumulated in PSUM over the 4 s-chunks of 128, interleaved with the
    mix chunk by chunk.  f32r runs the PE at 1 cycle/row (4x faster than
    fp32) at ~1e-4 relative error.
  - Bulk loads alternate between the two HWDGE rings (SP and ACT
    sequencers); fine-grained 384 KB loads keep completions flowing so
    compute starts early.  PSUM -> SBUF copy on ACT/DVE, DMA out.
"""

import numpy as np

L, B, S, F = 4, 32, 512, 768
W_MAX = 256
NW = W_MAX + 1  # 257
NCORES = 8
NB = B // NCORES  # sentences per core
P = 128
SC = S // P  # s-chunks per sentence

_module_cache: dict = {}


def _build_module(r0: float, r1: float, r2: float, col0: float, order):
    import concourse.bacc as bacc
    import concourse.bass as bass
    import concourse.mybir as mybir
    import concourse.tile as tile

    f32 = mybir.dt.float32
    f32r = mybir.dt.float32r
    mult = mybir.AluOpType.mult
    add = mybir.AluOpType.add

    nc = bacc.Bacc(
        "TRN2", target_bir_lowering=False, debug=False, num_devices=NCORES
    )
    hid = nc.dram_tensor("hid", (L, NB, S, F), f32, kind="ExternalInput").ap()
    mm = nc.dram_tensor("mm", (NB, P, SC, NW), f32r, kind="ExternalInput").ap()
    out = nc.dram_tensor("out", (NB, NW, F), f32, kind="ExternalOutput").ap()

    wtiles = [(0, 128), (128, 256), (256, 257)]
    fsplits = [(0, 384), (384, 768)]

    with tile.TileContext(nc) as tc:
        with (
            tc.tile_pool(name="const", bufs=1) as cpool,
            tc.tile_pool(name="h", bufs=20) as hpool,
            tc.tile_pool(name="t", bufs=4) as tpool,
            tc.tile_pool(name="sub", bufs=8) as spool,
            tc.tile_pool(name="m", bufs=4) as mpool,
            tc.tile_pool(name="o", bufs=8) as opool,
            tc.tile_pool(name="ps", bufs=8, space=bass.MemorySpace.PSUM) as pspool,
        ):
            mts = []
            for b in range(NB):
                mt = mpool.tile([P, SC, NW], f32r, tag="m", name=f"mt{b}")
                nc.sync.dma_start(mt[:], mm[b])
                mts.append(mt)
            for b in range(NB):
                mcs = [mts[b][:, c, :] for c in range(SC)]
                ps_tiles = {}
                for t in range(len(wtiles)):
                    for fi in range(len(fsplits)):
                        ps_tiles[t, fi] = pspool.tile(
                            [P, 384], f32, tag="ps", name=f"ps{b}_{t}_{fi}"
                        )
                for c in range(SC):
                    hts = []
                    for l in range(L):
                        ht = hpool.tile([P, F], f32, tag="h", name=f"h{b}_{c}_{l}")
                        # alternate between the two HWDGE rings (SP / ACT)
                        eng = nc.sync if (c * L + l) % 2 == 0 else nc.scalar
                        eng.dma_start(ht[:], hid[l, b, c * P : (c + 1) * P, :])
                        hts.append(ht)
                    ia, ib, ic, id_ = order
                    t1 = tpool.tile([P, F], f32, tag="t")
                    nc.vector.scalar_tensor_tensor(
                        t1[:], hts[ia][:], float(r0), hts[id_][:],
                        op0=mult, op1=add,
                    )
                    t2 = tpool.tile([P, F], f32, tag="t")
                    nc.vector.scalar_tensor_tensor(
                        t2[:], hts[ib][:], float(r1), hts[ic][:],
                        op0=mult, op1=add,
                    )
                    sub = spool.tile([P, F], f32r, tag="sub")
                    nc.vector.scalar_tensor_tensor(
                        sub[:], t2[:], float(r2), t1[:], op0=mult, op1=add
                    )
                    # accumulate this chunk into all word-tiles right away
                    for t, (w0, w1) in enumerate(wtiles):
                        msz = w1 - w0
                        for fi, (f0, f1) in enumerate(fsplits):
                            nc.tensor.matmul(
                                ps_tiles[t, fi][0:msz, 0 : f1 - f0],
                                mcs[c][:, w0:w1],
                                sub[:, f0:f1],
                                start=(c == 0),
                                stop=(c == SC - 1),
                            )
                for t, (w0, w1) in enumerate(wtiles):
                    msz = w1 - w0
                    ob = opool.tile([P, F], f32, tag="o")
                    for fi, (f0, f1) in enumerate(fsplits):
                        nc.any.tensor_copy(
                            ob[0:msz, f0:f1], ps_tiles[t, fi][0:msz, :]
                        )
                    nc.scalar.dma_start(out[b, w0:w1, :], ob[0:msz, :])

    nc.compile()
    return nc


def _prepare(hidden_states, layer_weights, gamma, word_ids):
    """Host-side prep: softmax ratios + per-position recip table + shards."""
    hidden_states = np.ascontiguousarray(hidden_states, dtype=np.float32)
    lw = np.asarray(layer_weights, dtype=np.float64)
    g = float(np.asarray(gamma, dtype=np.float64).reshape(-1)[0])
    ids = np.asarray(word_ids)

    e = np.exp(lw - lw.max())
    w = e / e.sum()  # softmax, float64
    # pair layers sorted by weight so every folded ratio is <= 1:
    #   sub*w[d] = w[a]h[a] + w[b]h[b] + w[c]h[c] + w[d]h[d]
    order = tuple(int(i) for i in np.argsort(w))
    ia, ib, ic, id_ = order
    r0 = float(w[ia] / w[id_])
    r1 = float(w[ib] / w[ic]) if w[ic] > 0 else 0.0
    r2 = float(w[ic] / w[id_])
    scale = float(w[id_] * g)  # absorbed into M
    col0 = float(np.float32(scale / S))

    mmat = np.zeros((B, S, NW), dtype=np.float64)
    rows = np.arange(S)
    for b in range(B):
        counts = np.bincount(ids[b], minlength=NW).astype(np.float64)
        recip = np.zeros(NW, dtype=np.float64)
        nz = counts > 0
        recip[nz] = scale / counts[nz]
        sel = ids[b] > 0
        mmat[b, rows[sel], ids[b][sel]] = recip[ids[b][sel]]
        mmat[b, :, 0] = scale / S
    mmat = mmat.reshape(B, SC, P, NW).transpose(0, 2, 1, 3)
    mmat = np.ascontiguousarray(mmat, dtype=np.float32)

    in_maps = []
    for i in range(NCORES):
        bs = slice(i * NB, (i + 1) * NB)
        in_maps.append(
            {
                "hid": np.ascontiguousarray(hidden_states[:, bs]),
                "mm": np.ascontiguousarray(mmat[bs]),
            }
        )
    return (r0, r1, r2, col0, order), in_maps


def _run(inputs: dict, trace: bool = False):
    from concourse.bass_utils import run_bass_kernel_spmd

    params, in_maps = _prepare(**inputs)
    if params not in _module_cache:
        _module_cache[params] = _build_module(*params)
    nc = _module_cache[params]

    res = run_bass_kernel_spmd(
        nc, in_maps, core_ids=list(range(NCORES)), trace=trace
    )
    out = np.concatenate([r["out"] for r in res.results], axis=0)
    return out, res


def kernel(**inputs) -> np.ndarray:
    out, _ = _run(inputs, trace=False)
    return out

